# revision 8
# baseline (speedup 1.0000x reference)
"""Trainium2 Bass kernel for nn_MultiHeadAttention_85375359909998.

Causal MHA with (non-standard interleaved) RoPE, fp32 in/out.
  B=2, T=2048, D=1024, H=16, DH=64.

Sharding over 8 NeuronCores: data-parallel over batch (2) x tensor-parallel
over head groups (16 heads -> 4 groups of 4). Each core computes its batch's
QKV projection for its 4 heads, RoPE, causal attention, and a partial output
projection; the host sums the 4 partial projections per batch (the
"all-reduce") and concatenates batches.

v2 layout notes (per core, heads grouped in pairs):
  - All HBM inputs arrive as ~9 large multi-block strided DMAs (1MB-class)
    split across the two HWDGE queues (scalar: xT; sync: weights/tables);
    output is fp16, staged per t-strip and stored as one 512KB DMA.
  - Scores accumulate into fp16 PSUM tiles [128, 2048] (2 banks per
    double-s-chunk group); exp reads a narrowed 4D AP (skips fully-masked
    columns); causal boundary masking is a post-exp multiply by a 0/1
    triangular mask (DVE/GpSimd), so no PSUM accumulation is needed.
  - The K=64 score matmuls for the two heads of a pair are emitted
    back-to-back with lhsT/rhs base partitions 0/64 so they run as
    concurrent PE row-tiles; the RoPE rotate matmul is similarly split
    into two K=64 row-tiles (R is block-diagonal).
  - Projection of strip j+1 is interleaved with attention of strip j in
    the emission order so the in-order PE queue stays dense (HAM warm)
    and eviction stalls hide under attention matmuls.
"""

import sys
from contextlib import ExitStack

import numpy as np

try:
    import concourse.bass as bass  # noqa: F401
except ImportError:  # pragma: no cover
    sys.path.insert(0, "/opt/trn_rl_repo")
    import concourse.bass as bass  # noqa: F401

import concourse.tile as tile
from concourse import bacc, mybir
from concourse import bass_utils

B, T, D, H, DH = 2, 2048, 1024, 16, 64
NCORES = 8
GROUPS = 4          # head groups (tensor-parallel dimension)
HPC = H // GROUPS   # 4 heads per core
NPAIR = HPC // 2    # head pairs per core
TC512 = T // 512    # 4 t-strips
SC128 = T // 128    # 16 s-chunks
KC = D // 128       # 8 contraction chunks for the projections
NEG8 = np.float32(-4.0e4)

f32 = mybir.dt.float32
f16 = mybir.dt.float16
EXP = mybir.ActivationFunctionType.Exp
COPY = mybir.ActivationFunctionType.Copy

_CACHE = {}


# --------------------------------------------------------------------------
# host-side constant tables
# --------------------------------------------------------------------------

def _rope_tables():
    """cos/sin tables, transposed & stacked for the [2*64, t] chunk layout."""
    inv = 1.0 / (10000.0 ** (np.arange(0, DH, 2, dtype=np.float64) / DH))
    t = np.arange(T, dtype=np.float64)
    freqs = t[:, None] * inv[None, :]                 # [T, 32]
    emb = np.concatenate([freqs, freqs], axis=-1)     # [T, 64]
    cos = np.cos(emb).astype(np.float32).T            # [64, T]
    sin = np.sin(emb).astype(np.float32).T
    csc = np.concatenate([cos, cos], axis=0)          # [128, T]
    csn = np.concatenate([sin, sin], axis=0)
    return (np.ascontiguousarray(csc.astype(np.float16)),
            np.ascontiguousarray(csn.astype(np.float16)))


def _rot64T():
    """R64.T stacked twice: (R@v)[2i] = -v[2i+1], (R@v)[2i+1] = v[2i]."""
    R = np.zeros((DH, DH), dtype=np.float32)
    for i in range(DH // 2):
        R[2 * i, 2 * i + 1] = -1.0
        R[2 * i + 1, 2 * i] = 1.0
    RT = np.ascontiguousarray(R.T)
    return np.concatenate([RT, RT], axis=0)  # [128, 64]


def _mask256():
    """[zeros(128) | upper-incl-tri01(128)] fp16 multiplicative mask."""
    sp = np.arange(128)[:, None]
    tp = np.arange(128)[None, :]
    tri = np.where(tp >= sp, np.float32(1.0), np.float32(0.0))
    return np.concatenate([np.zeros((128, 128), np.float32), tri],
                          axis=1).astype(np.float16)


# --------------------------------------------------------------------------
# device kernel
# --------------------------------------------------------------------------

def _emit(nc, tc, d, ctx):
    const = ctx.enter_context(tc.tile_pool(name="const", bufs=1))
    qkp = ctx.enter_context(tc.tile_pool(name="qk", bufs=1))
    vtp = ctx.enter_context(tc.tile_pool(name="vt", bufs=1))
    vp = ctx.enter_context(tc.tile_pool(name="v", bufs=1))
    att = ctx.enter_context(tc.tile_pool(name="att", bufs=1))
    ptp = ctx.enter_context(tc.tile_pool(name="pt", bufs=3))
    tmp = ctx.enter_context(tc.tile_pool(name="tmp", bufs=3))
    small = ctx.enter_context(tc.tile_pool(name="small", bufs=2))
    stage = ctx.enter_context(tc.tile_pool(name="stage", bufs=2))

    psA = ctx.enter_context(tc.tile_pool(name="psA", bufs=2, space="PSUM"))
    psS = ctx.enter_context(tc.tile_pool(name="psS", bufs=2, space="PSUM"))
    psO = ctx.enter_context(tc.tile_pool(name="psO", bufs=2, space="PSUM"))

    # ---- constants: few large DMAs, split across the two HWDGE queues ----
    # sync queue: weights + tables (wqk first: needed by the first matmul)
    wqk_t = const.tile([128, KC * 512], f16, tag="wqk")     # [128, 4096]
    nc.sync.dma_start(wqk_t[:], d["wqk"][:])
    wv_t = const.tile([128, KC * 256], f16, tag="wv")       # [128, 2048]
    nc.sync.dma_start(wv_t[:], d["wv"][:])
    msc_t = const.tile([128, 64 + 128 + 256 + 1], f16, tag="msc")
    nc.sync.dma_start(msc_t[:], d["msc"][:])
    rT_t = msc_t[:, 0:64]          # R64.T stacked twice on partitions
    id_t = msc_t[:, 64:192]        # identity (PE transpose)
    m256_t = msc_t[:, 192:448]     # [zeros | tri01] post-exp mask
    oc_t = msc_t[:, 448:449]       # ones column
    cs_t = const.tile([128, 2 * T], f16, tag="cs")          # [cos | sin]
    nc.sync.dma_start(cs_t[:], d["cs"][:])
    csc_t = cs_t[:, 0:T]
    csn_t = cs_t[:, T:2 * T]
    wp_t = const.tile([128, 2 * D], f16, tag="wp")          # [128, 2048]
    nc.sync.dma_start(wp_t[:], d["wp"][:])

    # scalar queue: x^T in t-strip chunks (strip 0 first, then the rest)
    xT_t = const.tile([128, KC * T], f16, tag="xT")         # [128, 16384]
    xsrc = d["xT"][:].rearrange("p (k t) -> p k t", k=KC, t=T)
    xdst = xT_t[:].rearrange("p (k t) -> p k t", k=KC, t=T)
    nc.scalar.dma_start(xdst[:, :, 0:512], xsrc[:, :, 0:512])
    for q in range(1, TC512):
        nc.scalar.dma_start(xdst[:, :, 512 * q:512 * (q + 1)],
                            xsrc[:, :, 512 * q:512 * (q + 1)])

    # ---- persistent activations ----
    qk = [qkp.tile([128, T], f16, tag=f"qk{j}", name=f"qk{j}")
          for j in range(4)]
    vt = [vtp.tile([128, T], f16, tag=f"vt{j}", name=f"vt{j}")
          for j in range(2)]
    v_sb = [vp.tile([128, 4 * 65], f16, tag=f"v{i}", name=f"v{i}")
            for i in range(SC128)]
    attn = [att.tile([128, T], f16, tag=f"at{p}", name=f"at{p}")
            for p in range(NPAIR)]

    # ------------------------------------------------------------------
    # emission units
    # ------------------------------------------------------------------

    def proj_units(tcc):
        """Projection + RoPE + V-transpose for t-strip tcc, as a list of
        small emission callables (interleaved with attention of the
        previous strip by the driver)."""
        tsl = slice(512 * tcc, 512 * (tcc + 1))
        units = []

        def chain(jc):
            def emit():
                ps = psA.tile([128, 512], f32, tag="ps")
                for kc in range(KC):
                    if jc < 4:
                        lhsT = wqk_t[:, 512 * kc + 128 * jc:
                                     512 * kc + 128 * (jc + 1)]
                    else:
                        lhsT = wv_t[:, 256 * kc + 128 * (jc - 4):
                                    256 * kc + 128 * (jc - 3)]
                    nc.tensor.matmul(ps[:], lhsT,
                                     xT_t[:, T * kc + 512 * tcc:
                                          T * kc + 512 * (tcc + 1)],
                                     start=(kc == 0), stop=(kc == KC - 1))
                dst = (qk[jc][:, tsl] if jc < 4 else vt[jc - 4][:, tsl])
                nc.scalar.copy(dst, ps[:])
            return emit

        def rope(jc):
            def emit():
                dst = qk[jc][:, tsl]
                rps = psA.tile([128, 512], f32, tag="ps")
                # block-diagonal rotate as two concurrent K=64 row-tiles
                nc.tensor.matmul(rps[0:64, :], rT_t[0:64, :], dst[0:64, :],
                                 start=True, stop=True,
                                 tile_position=(0, 0))
                nc.tensor.matmul(rps[64:128, :], rT_t[64:128, :],
                                 dst[64:128, :], start=True, stop=True,
                                 tile_position=(64, 64))
                t1 = tmp.tile([128, 512], f32, tag="t1")
                nc.vector.tensor_mul(t1[:], rps[:], csn_t[:, tsl])
                t2 = tmp.tile([128, 512], f32, tag="t2")
                nc.gpsimd.tensor_mul(t2[:], dst, csc_t[:, tsl])
                nc.vector.tensor_add(dst, t1[:], t2[:])
            return emit

        def vtrans(b4):
            def emit():
                i = 4 * tcc + b4
                vdst = v_sb[i]
                for vc in range(2):
                    pst = psA.tile([128, 128], f16, tag="ps")
                    nc.tensor.transpose(
                        pst[:], vt[vc][:, 128 * i:128 * (i + 1)], id_t)
                    ha = 2 * vc
                    dst3 = vdst[:, 65 * ha:65 * ha + 130].rearrange(
                        "p (h e) -> p h e", h=2, e=65)[:, :, 0:64]
                    src3 = pst[:].rearrange("p (h e) -> p h e", h=2, e=64)
                    nc.vector.tensor_copy(dst3, src3)
                nc.vector.tensor_copy(vdst[:, 64:260:65],
                                      oc_t.broadcast_to([128, 4]))
            return emit

        # q/k first (rope depends on them), interleave rope, then v + trans
        units.append(chain(0))
        units.append(chain(1))
        units.append(rope(0))
        units.append(chain(2))
        units.append(rope(1))
        units.append(chain(3))
        units.append(rope(2))
        units.append(chain(4))
        units.append(rope(3))
        units.append(chain(5))
        for b4 in range(4):
            units.append(vtrans(b4))
        return units

    def attn_units(j):
        """Attention + out-projection for t-strip j as emission callables."""
        tsl = slice(512 * j, 512 * (j + 1))
        ni = 4 * (j + 1)
        units = []
        po_ref = {}

        def group(p, ii):
            def emit():
                qc = qk[2 * p]
                kch = qk[2 * p + 1]
                diag = ii >= 4 * j
                c0 = 128 * (ii - 4 * j) if diag else 0
                qs = slice(512 * j + c0, 512 * (j + 1))
                # scores for one s-chunk, both heads of the pair: the two
                # K=64 matmuls use base partitions 0/64 so they run as
                # concurrent PE row-tiles
                pss = psS.tile([128, 1024], f32, tag="s")
                for hh in range(2):
                    hsl = slice(64 * hh, 64 * (hh + 1))
                    nc.tensor.matmul(
                        pss[:, 512 * hh + c0:512 * (hh + 1)],
                        kch[hsl, 128 * ii:128 * (ii + 1)],
                        qc[hsl, qs], start=True, stop=True)
                ptl = ptp.tile([128, 1024], f16, tag="ptl")
                src3 = pss[:].rearrange("p (b c) -> p b c", b=2,
                                        c=512)[:, :, c0:512]
                dst3 = ptl[:].rearrange("p (b c) -> p b c", b=2,
                                        c=512)[:, :, c0:512]
                nc.scalar.activation(dst3, src3, EXP, scale=0.125)
                if diag:
                    # post-exp multiplicative causal mask on the boundary
                    # block [c0, c0+128) of both heads
                    d0 = ptl[:].rearrange("p (b c) -> p b c", b=2,
                                          c=512)[:, :, c0:c0 + 128]
                    m0 = m256_t[:, 128:256].rearrange(
                        "p (x c) -> p x c", x=1).broadcast_to([128, 2, 128])
                    nc.vector.tensor_mul(d0, d0, m0)
                if ii == 0:
                    po_ref[p] = [psO.tile([65, 512], f32, tag="acc",
                                          name=f"po{hh}") for hh in range(2)]
                po = po_ref[p]
                for hh in range(2):
                    h = 2 * p + hh
                    nc.tensor.matmul(
                        po[hh][:, c0:512],
                        v_sb[ii][:, 65 * h:65 * h + 65],
                        ptl[:, 512 * hh + c0:512 * (hh + 1)],
                        start=(ii == 0), stop=(ii == ni - 1))
            return emit

        def norm(p):
            def emit():
                po = po_ref[p]
                for hh in range(2):
                    sr = small.tile([1, 512], f32, tag="sr")
                    nc.vector.tensor_copy(sr[:], po[hh][64:65, :])
                    rc = small.tile([1, 512], f32, tag="rc")
                    nc.vector.reciprocal_approx_fast(rc[:], sr[:])
                    bsb = small.tile([64, 512], f32, tag="bsb")
                    nc.gpsimd.partition_broadcast(bsb[:], rc[0:1, :])
                    nc.vector.tensor_mul(
                        attn[p][64 * hh:64 * (hh + 1), tsl],
                        po[hh][0:64, :], bsb[:])
            return emit

        ob_ref = {}

        def outproj(oc):
            def emit():
                if oc == 0:
                    ob_ref[0] = stage.tile([128, 8 * 512], f16, tag="ob",
                                           name="ob")
                pp = psO.tile([128, 512], f32, tag="acc", name="pp")
                for kc2 in range(2):
                    nc.tensor.matmul(
                        pp[:], wp_t[:, D * kc2 + 128 * oc:
                                    D * kc2 + 128 * (oc + 1)],
                        attn[kc2][:, tsl], start=(kc2 == 0), stop=(kc2 == 1))
                ob = ob_ref[0]
                if oc % 2 == 0:
                    nc.vector.tensor_copy(ob[:, 512 * oc:512 * (oc + 1)],
                                          pp[:])
                else:
                    nc.scalar.activation(ob[:, 512 * oc:512 * (oc + 1)],
                                         pp[:], COPY)
                if oc == 7:
                    osrc = ob[:].rearrange("p (o t) -> p o t", o=8, t=512)
                    odst = d["o"][:].rearrange(
                        "(o p) t -> p o t", o=8, p=128)[:, :, tsl]
                    nc.sync.dma_start(odst, osrc)
            return emit

        for ii in range(ni):
            units.append(group(0, ii))
        units.append(norm(0))
        for ii in range(ni):
            units.append(group(1, ii))
        units.append(norm(1))
        for oc in range(8):
            units.append(outproj(oc))
        return units

    # ------------------------------------------------------------------
    # driver: strip 0 projection, then attention(j) interleaved with
    # projection(j+1)
    # ------------------------------------------------------------------
    for u in proj_units(0):
        u()
    for j in range(TC512):
        au = attn_units(j)
        pu = proj_units(j + 1) if j + 1 < TC512 else []
        # round-robin: spread projection units across attention units
        na, npu = len(au), len(pu)
        pi = 0
        for k, u in enumerate(au):
            u()
            want = (k + 1) * npu // na
            while pi < want:
                pu[pi]()
                pi += 1
        while pi < npu:
            pu[pi]()
            pi += 1


def _build_module():
    nc = bacc.Bacc("TRN2", target_bir_lowering=False, debug=False,
                   enable_asserts=False)
    d = {
        "xT": nc.dram_tensor("xT", [128, KC * T], f16,
                             kind="ExternalInput").ap(),
        "wqk": nc.dram_tensor("wqk", [128, KC * 512], f16,
                              kind="ExternalInput").ap(),
        "wv": nc.dram_tensor("wv", [128, KC * 256], f16,
                             kind="ExternalInput").ap(),
        "wp": nc.dram_tensor("wp", [128, 2 * D], f16,
                             kind="ExternalInput").ap(),
        "cs": nc.dram_tensor("cs", [128, 2 * T], f16,
                             kind="ExternalInput").ap(),
        "msc": nc.dram_tensor("msc", [128, 449], f16,
                              kind="ExternalInput").ap(),
        "o": nc.dram_tensor("o", [D, T], f16, kind="ExternalOutput").ap(),
    }
    with tile.TileContext(nc) as tc:
        with ExitStack() as ctx, \
             nc.allow_low_precision("fp16 PE operands/psum are by design"):
            _emit(nc, tc, d, ctx)
    nc.compile()
    return nc


def _get_module():
    if "nc" not in _CACHE:
        _CACHE["nc"] = _build_module()
    return _CACHE["nc"]


# --------------------------------------------------------------------------
# host wrapper
# --------------------------------------------------------------------------

def _canonical(attn_mask, key_padding_mask):
    if attn_mask.shape != (1, 1, T, T) or key_padding_mask.shape != (B, T):
        return False
    if not key_padding_mask.all():
        return False
    m = np.asarray(attn_mask[0, 0], dtype=np.float32)
    causal = np.triu(np.full((T, T), -1e9, dtype=np.float32), k=1)
    return np.array_equal(m, causal)


def _reference_fallback(x, attn_mask, key_padding_mask, Wqkv, Wproj):
    x = np.asarray(x, np.float32)
    qkv = x @ np.asarray(Wqkv, np.float32).T
    q, k, v = qkv[..., :D], qkv[..., D:2 * D], qkv[..., 2 * D:]

    def split(t):
        return t.reshape(B, -1, H, DH).transpose(0, 2, 1, 3)

    def rope(xx):
        inv = 1.0 / (10000.0 ** (np.arange(0, DH, 2, dtype=np.float32) / DH))
        fr = np.arange(T, dtype=np.float32)[:, None] * inv[None, :]
        emb = np.concatenate([fr, fr], axis=-1)
        cos, sin = np.cos(emb)[None, None], np.sin(emb)[None, None]
        x1, x2 = xx[..., ::2], xx[..., 1::2]
        rh = np.stack((-x2, x1), axis=-1).reshape(xx.shape)
        return xx * cos + rh * sin

    q, k, v = split(q), split(k), split(v)
    q, k = rope(q), rope(k)
    s = np.einsum("bhtd,bhsd->bhts", q, k) / np.sqrt(np.float32(DH))
    s = s + np.asarray(attn_mask, np.float32)
    s = np.where(np.asarray(key_padding_mask)[:, None, None, :], s, -1e9)
    s = s - s.max(axis=-1, keepdims=True)
    e = np.exp(s)
    a = e / e.sum(axis=-1, keepdims=True)
    out = np.einsum("bhts,bhsd->bhtd", a, v)
    out = out.transpose(0, 2, 1, 3).reshape(B, T, D)
    return out @ np.asarray(Wproj, np.float32).T


def _kc_fold(a):
    """[D, N] -> [128, KC*N] with kc-major column blocks."""
    n = a.shape[1]
    return np.ascontiguousarray(
        a.reshape(KC, 128, n).transpose(1, 0, 2).reshape(128, KC * n))


def _make_in_maps(x, Wqkv, Wproj):
    csc, csn = _rope_tables()
    cs = np.ascontiguousarray(np.concatenate([csc, csn], axis=1))
    rT2 = _rot64T().astype(np.float16)                       # [128, 64]
    ident = np.eye(128, dtype=np.float16)
    m256 = _mask256()                                        # [128, 256]
    onesc = np.ones((128, 1), dtype=np.float16)
    msc = np.ascontiguousarray(
        np.concatenate([rT2, ident, m256, onesc], axis=1))   # [128, 449]

    Wq = np.asarray(Wqkv[:D], np.float32).reshape(H, DH, D)
    Wk = np.asarray(Wqkv[D:2 * D], np.float32).reshape(H, DH, D)
    Wv = np.asarray(Wqkv[2 * D:], np.float32).reshape(H, DH, D)
    WpT = np.ascontiguousarray(np.asarray(Wproj, np.float32).T)

    xT = []
    for b in range(B):
        xt = np.asarray(x[b], np.float32).T.astype(np.float16)  # [D, T]
        xT.append(_kc_fold(xt))

    in_maps = []
    for c in range(NCORES):
        b, g = divmod(c, GROUPS)
        hs = [HPC * g + hl for hl in range(HPC)]
        cols = []
        for pp in range(NPAIR):
            h0, h1 = hs[2 * pp], hs[2 * pp + 1]
            cols.append(np.concatenate([Wq[h0], Wq[h1]], axis=0))
            cols.append(np.concatenate([Wk[h0], Wk[h1]], axis=0))
        wqk = _kc_fold(np.concatenate(cols, axis=0).T.astype(np.float16))
        wv = _kc_fold(np.concatenate(
            [Wv[h] for h in hs], axis=0).T.astype(np.float16))
        wp = np.ascontiguousarray(
            WpT[256 * g:256 * (g + 1), :].astype(np.float16))
        wp = np.ascontiguousarray(
            wp.reshape(2, 128, D).transpose(1, 0, 2).reshape(128, 2 * D))
        in_maps.append({
            "xT": xT[b], "wqk": wqk, "wv": wv, "wp": wp,
            "cs": cs, "msc": msc,
        })
    return in_maps


def _in_maps_for_trace(inputs):
    return _make_in_maps(np.asarray(inputs["x"]), np.asarray(inputs["Wqkv"]),
                         np.asarray(inputs["Wproj"]))


def kernel(x, attn_mask, key_padding_mask, Wqkv, Wproj):
    x = np.asarray(x)
    attn_mask = np.asarray(attn_mask)
    key_padding_mask = np.asarray(key_padding_mask)
    Wqkv = np.asarray(Wqkv)
    Wproj = np.asarray(Wproj)

    if not _canonical(attn_mask, key_padding_mask):
        return _reference_fallback(x, attn_mask, key_padding_mask, Wqkv, Wproj)

    nc = _get_module()
    in_maps = _make_in_maps(x, Wqkv, Wproj)
    res = bass_utils.run_bass_kernel_spmd(nc, in_maps,
                                          core_ids=list(range(NCORES)))
    out = np.empty((B, T, D), dtype=np.float32)
    for b in range(B):
        acc = res.results[4 * b]["o"].astype(np.float32)
        for g in range(1, GROUPS):
            acc += res.results[4 * b + g]["o"].astype(np.float32)
        out[b] = acc.T
    return out


# revision 17
# speedup vs baseline: 1.5271x; 1.5271x over previous
"""Trainium2 Bass kernel for nn_MultiHeadAttention_85375359909998.

Causal MHA with (non-standard interleaved) RoPE, fp32 in/out.
  B=2, T=2048, D=1024, H=16, DH=64.

Sharding over 8 NeuronCores: data-parallel over batch (2) x tensor-parallel
over head groups (16 heads -> 4 groups of 4). Each core computes its batch's
QKV projection for its 4 heads, RoPE, causal attention, and a partial output
projection; the host sums the 4 partial projections per batch (the
"all-reduce") and concatenates batches.

v2 layout notes (per core, heads grouped in pairs):
  - All HBM inputs arrive as ~9 large multi-block strided DMAs (1MB-class)
    split across the two HWDGE queues (scalar: xT; sync: weights/tables);
    output is fp16, staged per t-strip and stored as one 512KB DMA.
  - Scores accumulate into fp16 PSUM tiles [128, 2048] (2 banks per
    double-s-chunk group); exp reads a narrowed 4D AP (skips fully-masked
    columns); causal boundary masking is a post-exp multiply by a 0/1
    triangular mask (DVE/GpSimd), so no PSUM accumulation is needed.
  - The K=64 score matmuls for the two heads of a pair are emitted
    back-to-back with lhsT/rhs base partitions 0/64 so they run as
    concurrent PE row-tiles; the RoPE rotate matmul is similarly split
    into two K=64 row-tiles (R is block-diagonal).
  - Projection of strip j+1 is interleaved with attention of strip j in
    the emission order so the in-order PE queue stays dense (HAM warm)
    and eviction stalls hide under attention matmuls.
"""

import sys
from contextlib import ExitStack

import numpy as np

try:
    import concourse.bass as bass  # noqa: F401
except ImportError:  # pragma: no cover
    sys.path.insert(0, "/opt/trn_rl_repo")
    import concourse.bass as bass  # noqa: F401

import concourse.tile as tile
from concourse import bacc, mybir
from concourse import bass_utils

B, T, D, H, DH = 2, 2048, 1024, 16, 64
NCORES = 8
GROUPS = 4          # head groups (tensor-parallel dimension)
HPC = H // GROUPS   # 4 heads per core
NPAIR = HPC // 2    # head pairs per core
TC512 = T // 512    # 4 t-strips
SC128 = T // 128    # 16 s-chunks
KC = D // 128       # 8 contraction chunks for the projections
NEG8 = np.float32(-4.0e4)

f32 = mybir.dt.float32
f16 = mybir.dt.float16
EXP = mybir.ActivationFunctionType.Exp
COPY = mybir.ActivationFunctionType.Copy

_CACHE = {}


# --------------------------------------------------------------------------
# host-side constant tables
# --------------------------------------------------------------------------

def _rope_tables():
    """cos/sin tables, transposed & stacked for the [2*64, t] chunk layout."""
    inv = 1.0 / (10000.0 ** (np.arange(0, DH, 2, dtype=np.float64) / DH))
    t = np.arange(T, dtype=np.float64)
    freqs = t[:, None] * inv[None, :]                 # [T, 32]
    emb = np.concatenate([freqs, freqs], axis=-1)     # [T, 64]
    cos = np.cos(emb).astype(np.float32).T            # [64, T]
    sin = np.sin(emb).astype(np.float32).T
    csc = np.concatenate([cos, cos], axis=0)          # [128, T]
    csn = np.concatenate([sin, sin], axis=0)
    return (np.ascontiguousarray(csc.astype(np.float16)),
            np.ascontiguousarray(csn.astype(np.float16)))


def _rot64T():
    """R64.T stacked twice: (R@v)[2i] = -v[2i+1], (R@v)[2i+1] = v[2i]."""
    R = np.zeros((DH, DH), dtype=np.float32)
    for i in range(DH // 2):
        R[2 * i, 2 * i + 1] = -1.0
        R[2 * i + 1, 2 * i] = 1.0
    RT = np.ascontiguousarray(R.T)
    return np.concatenate([RT, RT], axis=0)  # [128, 64]


def _tri_neg():
    """Strict lower-tri NEG8 additive mask for the diagonal boundary."""
    sp = np.arange(128)[:, None]
    tp = np.arange(128)[None, :]
    return np.where(tp < sp, NEG8, np.float32(0.0)).astype(np.float16)


# --------------------------------------------------------------------------
# device kernel
# --------------------------------------------------------------------------

def _emit(nc, tc, d, ctx):
    const = ctx.enter_context(tc.tile_pool(name="const", bufs=1))
    qkp = ctx.enter_context(tc.tile_pool(name="qk", bufs=1))
    vtp = ctx.enter_context(tc.tile_pool(name="vt", bufs=1))
    vp = ctx.enter_context(tc.tile_pool(name="v", bufs=1))
    att = ctx.enter_context(tc.tile_pool(name="att", bufs=1))
    ptp = ctx.enter_context(tc.tile_pool(name="pt", bufs=3))
    tmp = ctx.enter_context(tc.tile_pool(name="tmp", bufs=3))
    small = ctx.enter_context(tc.tile_pool(name="small", bufs=2))
    stage = ctx.enter_context(tc.tile_pool(name="stage", bufs=2))

    psA = ctx.enter_context(tc.tile_pool(name="psA", bufs=2, space="PSUM"))
    psS = ctx.enter_context(tc.tile_pool(name="psS", bufs=2, space="PSUM"))
    psO = ctx.enter_context(tc.tile_pool(name="psO", bufs=2, space="PSUM"))

    # ---- constants: few large DMAs, split across the two HWDGE queues ----
    # sync queue: weights + tables (wqk first: needed by the first matmul)
    wqk_t = const.tile([128, KC * 512], f16, tag="wqk")     # [128, 4096]
    nc.sync.dma_start(wqk_t[:], d["wqk"][:])
    wv_t = const.tile([128, KC * 256], f16, tag="wv")       # [128, 2048]
    nc.sync.dma_start(wv_t[:], d["wv"][:])
    msc_t = const.tile([128, 64 + 128 + 128 + 1], f16, tag="msc")
    nc.sync.dma_start(msc_t[:], d["msc"][:])
    rT_t = msc_t[:, 0:64]          # R64.T stacked twice on partitions
    id_t = msc_t[:, 64:192]        # identity (PE transpose + pattern)
    tri_t = msc_t[:, 192:320]      # strict-lower-tri NEG8 boundary mask
    oc_t = msc_t[:, 320:321]       # ones column
    cs_t = const.tile([128, 2 * T], f16, tag="cs")          # [cos | sin]
    nc.sync.dma_start(cs_t[:], d["cs"][:])
    csc_t = cs_t[:, 0:T]
    csn_t = cs_t[:, T:2 * T]
    wp_t = const.tile([128, 2 * D], f16, tag="wp")          # [128, 2048]
    nc.sync.dma_start(wp_t[:], d["wp"][:])

    # scalar queue: x^T strip-major (each t-strip contiguous: one 1MB DMA)
    xT_t = const.tile([128, KC * T], f16, tag="xT")         # [128, 16384]
    for q in range(TC512):
        nc.scalar.dma_start(xT_t[:, 4096 * q:4096 * (q + 1)],
                            d["xT"][:, 4096 * q:4096 * (q + 1)])

    # ---- persistent activations ----
    qk = [qkp.tile([128, T], f16, tag=f"qk{j}", name=f"qk{j}")
          for j in range(4)]
    vt = [vtp.tile([128, T], f16, tag=f"vt{j}", name=f"vt{j}")
          for j in range(2)]
    # one [s, dh] tile for all 16 s-chunks; per chunk: 4 heads x (64 v + 1
    # ones) columns.  The ones columns are written once up front.
    v_all = vp.tile([128, SC128 * 4 * 65], f16, tag="v", name="v_all")
    nc.vector.tensor_copy(v_all[:, 64:SC128 * 260:65],
                          oc_t.broadcast_to([128, SC128 * 4]))
    attn = [att.tile([128, T], f16, tag=f"at{p}", name=f"at{p}")
            for p in range(NPAIR)]

    # ------------------------------------------------------------------
    # emission units
    # ------------------------------------------------------------------

    def proj_units(tcc):
        """Projection + RoPE + V-transpose for t-strip tcc, as a list of
        small emission callables (interleaved with attention of the
        previous strip by the driver)."""
        tsl = slice(512 * tcc, 512 * (tcc + 1))
        units = []

        def chain(jc):
            def emit():
                ps = psA.tile([128, 512], f32, tag="ps")
                for kc in range(KC):
                    if jc < 4:
                        lhsT = wqk_t[:, 512 * kc + 128 * jc:
                                     512 * kc + 128 * (jc + 1)]
                    else:
                        lhsT = wv_t[:, 256 * kc + 128 * (jc - 4):
                                    256 * kc + 128 * (jc - 3)]
                    nc.tensor.matmul(ps[:], lhsT,
                                     xT_t[:, 4096 * tcc + 512 * kc:
                                          4096 * tcc + 512 * (kc + 1)],
                                     start=(kc == 0), stop=(kc == KC - 1))
                dst = (qk[jc][:, tsl] if jc < 4 else vt[jc - 4][:, tsl])
                nc.scalar.copy(dst, ps[:])
            return emit

        def rope(jc):
            def emit():
                dst = qk[jc][:, tsl]
                rps = psA.tile([128, 512], f32, tag="ps")
                # block-diagonal rotate as two concurrent K=64 row-tiles
                nc.tensor.matmul(rps[0:64, :], rT_t[0:64, :], dst[0:64, :],
                                 start=True, stop=True,
                                 tile_position=(0, 0))
                nc.tensor.matmul(rps[64:128, :], rT_t[64:128, :],
                                 dst[64:128, :], start=True, stop=True,
                                 tile_position=(64, 64))
                t1 = tmp.tile([128, 512], f32, tag="t1")
                nc.vector.tensor_mul(t1[:], rps[:], csn_t[:, tsl])
                t2 = tmp.tile([128, 512], f16, tag="t2")
                nc.vector.tensor_mul(t2[:], dst, csc_t[:, tsl])
                nc.vector.tensor_add(dst, t1[:], t2[:])
            return emit

        def vtrans(b4):
            def emit():
                i = 4 * tcc + b4
                for vc in range(2):
                    pst = psA.tile([128, 128], f16, tag="ps")
                    nc.tensor.transpose(
                        pst[:], vt[vc][:, 128 * i:128 * (i + 1)], id_t)
                    ha = 2 * vc
                    dst3 = v_all[:, 260 * i + 65 * ha:
                                 260 * i + 65 * ha + 130].rearrange(
                        "p (h e) -> p h e", h=2, e=65)[:, :, 0:64]
                    src3 = pst[:].rearrange("p (h e) -> p h e", h=2, e=64)
                    nc.vector.tensor_copy(dst3, src3)
            return emit

        # q/k first (rope depends on them), interleave rope, then v + trans
        units.append(chain(0))
        units.append(chain(1))
        units.append(rope(0))
        units.append(chain(2))
        units.append(rope(1))
        units.append(chain(3))
        units.append(rope(2))
        units.append(chain(4))
        units.append(rope(3))
        units.append(chain(5))
        for b4 in range(4):
            units.append(vtrans(b4))
        return units

    def attn_units(j):
        """Attention + out-projection for t-strip j as emission callables."""
        tsl = slice(512 * j, 512 * (j + 1))
        ni = 4 * (j + 1)
        units = []
        po_ref = {}

        def group(p, ii):
            def emit():
                qc = qk[2 * p]
                kch = qk[2 * p + 1]
                diag = ii >= 4 * j
                c0 = 128 * (ii - 4 * j) if diag else 0
                qs = slice(512 * j + c0, 512 * (j + 1))
                # scores for one s-chunk, both heads of the pair: the two
                # K=64 matmuls use base partitions 0/64 so they run as
                # concurrent PE row-tiles
                pss = psS.tile([128, 1024], f32, tag="s")
                for hh in range(2):
                    hsl = slice(64 * hh, 64 * (hh + 1))
                    nc.tensor.matmul(
                        pss[:, 512 * hh + c0:512 * (hh + 1)],
                        kch[hsl, 128 * ii:128 * (ii + 1)],
                        qc[hsl, qs], start=True, stop=not diag)
                if diag:
                    # accumulate the strict-lower-tri NEG8 boundary mask
                    for hh in range(2):
                        nc.tensor.matmul(
                            pss[:, 512 * hh + c0:512 * hh + c0 + 128],
                            id_t, tri_t, start=False, stop=True)
                ptl = ptp.tile([128, 1024], f16, tag="ptl")
                src3 = pss[:].rearrange("p (b c) -> p b c", b=2,
                                        c=512)[:, :, c0:512]
                dst3 = ptl[:].rearrange("p (b c) -> p b c", b=2,
                                        c=512)[:, :, c0:512]
                nc.scalar.activation(dst3, src3, EXP, scale=0.125)
                if ii == 0:
                    po_ref[p] = [psO.tile([65, 512], f32, tag="acc",
                                          name=f"po{hh}") for hh in range(2)]
                po = po_ref[p]
                for hh in range(2):
                    h = 2 * p + hh
                    nc.tensor.matmul(
                        po[hh][:, c0:512],
                        v_all[:, 260 * ii + 65 * h:260 * ii + 65 * h + 65],
                        ptl[:, 512 * hh + c0:512 * (hh + 1)],
                        start=(ii == 0), stop=(ii == ni - 1))
            return emit

        def norm(p):
            def emit():
                po = po_ref[p]
                # evict po to SBUF fast (frees the PSUM slots for the next
                # pair), then normalize SBUF-side off the PE critical path
                pov = tmp.tile([65, 1024], f32, tag="pov")
                nc.scalar.activation(pov[:, 0:512], po[0][:], COPY)
                nc.vector.tensor_copy(pov[:, 512:1024], po[1][:])
                rc = small.tile([1, 1024], f32, tag="rc")
                nc.vector.reciprocal_approx_fast(rc[:], pov[64:65, :])
                bsb = small.tile([64, 1024], f32, tag="bsb")
                nc.gpsimd.partition_broadcast(bsb[:], rc[0:1, :])
                for hh in range(2):
                    nc.vector.tensor_mul(
                        attn[p][64 * hh:64 * (hh + 1), tsl],
                        pov[0:64, 512 * hh:512 * (hh + 1)],
                        bsb[:, 512 * hh:512 * (hh + 1)])
            return emit

        ob_ref = {}

        def outproj(oc):
            def emit():
                if oc == 0:
                    ob_ref[0] = stage.tile([128, 8 * 512], f16, tag="ob",
                                           name="ob")
                pp = psO.tile([128, 512], f32, tag="acc", name="pp")
                for kc2 in range(2):
                    nc.tensor.matmul(
                        pp[:], wp_t[:, D * kc2 + 128 * oc:
                                    D * kc2 + 128 * (oc + 1)],
                        attn[kc2][:, tsl], start=(kc2 == 0), stop=(kc2 == 1))
                ob = ob_ref[0]
                if oc % 2 == 0:
                    nc.vector.tensor_copy(ob[:, 512 * oc:512 * (oc + 1)],
                                          pp[:])
                else:
                    nc.scalar.activation(ob[:, 512 * oc:512 * (oc + 1)],
                                         pp[:], COPY)
                if oc == 7:
                    # strip-major contiguous store; host reassembles
                    nc.sync.dma_start(d["o"][:, 4096 * j:4096 * (j + 1)],
                                      ob[:])
            return emit

        for ii in range(ni):
            units.append(group(0, ii))
        units.append(norm(0))
        for ii in range(ni):
            units.append(group(1, ii))
        units.append(norm(1))
        for oc in range(8):
            units.append(outproj(oc))
        return units

    # ------------------------------------------------------------------
    # driver: strip 0 projection, then attention(j) interleaved with
    # projection(j+1)
    # ------------------------------------------------------------------
    for u in proj_units(0):
        u()
    for j in range(TC512):
        au = attn_units(j)
        pu = proj_units(j + 1) if j + 1 < TC512 else []
        # round-robin: spread projection units across attention units
        na, npu = len(au), len(pu)
        pi = 0
        for k, u in enumerate(au):
            u()
            want = (k + 1) * npu // na
            while pi < want:
                pu[pi]()
                pi += 1
        while pi < npu:
            pu[pi]()
            pi += 1


def _build_module():
    nc = bacc.Bacc("TRN2", target_bir_lowering=False, debug=False,
                   enable_asserts=False)
    d = {
        "xT": nc.dram_tensor("xT", [128, KC * T], f16,
                             kind="ExternalInput").ap(),
        "wqk": nc.dram_tensor("wqk", [128, KC * 512], f16,
                              kind="ExternalInput").ap(),
        "wv": nc.dram_tensor("wv", [128, KC * 256], f16,
                             kind="ExternalInput").ap(),
        "wp": nc.dram_tensor("wp", [128, 2 * D], f16,
                             kind="ExternalInput").ap(),
        "cs": nc.dram_tensor("cs", [128, 2 * T], f16,
                             kind="ExternalInput").ap(),
        "msc": nc.dram_tensor("msc", [128, 321], f16,
                              kind="ExternalInput").ap(),
        "o": nc.dram_tensor("o", [128, 8 * T], f16,
                            kind="ExternalOutput").ap(),
    }
    with tile.TileContext(nc) as tc:
        with ExitStack() as ctx, \
             nc.allow_low_precision("fp16 PE operands/psum are by design"):
            _emit(nc, tc, d, ctx)
    nc.compile()
    return nc


def _get_module():
    if "nc" not in _CACHE:
        _CACHE["nc"] = _build_module()
    return _CACHE["nc"]


# --------------------------------------------------------------------------
# host wrapper
# --------------------------------------------------------------------------

def _canonical(attn_mask, key_padding_mask):
    if attn_mask.shape != (1, 1, T, T) or key_padding_mask.shape != (B, T):
        return False
    if not key_padding_mask.all():
        return False
    m = np.asarray(attn_mask[0, 0], dtype=np.float32)
    causal = np.triu(np.full((T, T), -1e9, dtype=np.float32), k=1)
    return np.array_equal(m, causal)


def _reference_fallback(x, attn_mask, key_padding_mask, Wqkv, Wproj):
    x = np.asarray(x, np.float32)
    qkv = x @ np.asarray(Wqkv, np.float32).T
    q, k, v = qkv[..., :D], qkv[..., D:2 * D], qkv[..., 2 * D:]

    def split(t):
        return t.reshape(B, -1, H, DH).transpose(0, 2, 1, 3)

    def rope(xx):
        inv = 1.0 / (10000.0 ** (np.arange(0, DH, 2, dtype=np.float32) / DH))
        fr = np.arange(T, dtype=np.float32)[:, None] * inv[None, :]
        emb = np.concatenate([fr, fr], axis=-1)
        cos, sin = np.cos(emb)[None, None], np.sin(emb)[None, None]
        x1, x2 = xx[..., ::2], xx[..., 1::2]
        rh = np.stack((-x2, x1), axis=-1).reshape(xx.shape)
        return xx * cos + rh * sin

    q, k, v = split(q), split(k), split(v)
    q, k = rope(q), rope(k)
    s = np.einsum("bhtd,bhsd->bhts", q, k) / np.sqrt(np.float32(DH))
    s = s + np.asarray(attn_mask, np.float32)
    s = np.where(np.asarray(key_padding_mask)[:, None, None, :], s, -1e9)
    s = s - s.max(axis=-1, keepdims=True)
    e = np.exp(s)
    a = e / e.sum(axis=-1, keepdims=True)
    out = np.einsum("bhts,bhsd->bhtd", a, v)
    out = out.transpose(0, 2, 1, 3).reshape(B, T, D)
    return out @ np.asarray(Wproj, np.float32).T


def _kc_fold(a):
    """[D, N] -> [128, KC*N] with kc-major column blocks."""
    n = a.shape[1]
    return np.ascontiguousarray(
        a.reshape(KC, 128, n).transpose(1, 0, 2).reshape(128, KC * n))


def _make_in_maps(x, Wqkv, Wproj):
    csc, csn = _rope_tables()
    cs = np.ascontiguousarray(np.concatenate([csc, csn], axis=1))
    rT2 = _rot64T().astype(np.float16)                       # [128, 64]
    ident = np.eye(128, dtype=np.float16)
    tri = _tri_neg()                                         # [128, 128]
    onesc = np.ones((128, 1), dtype=np.float16)
    msc = np.ascontiguousarray(
        np.concatenate([rT2, ident, tri, onesc], axis=1))    # [128, 321]

    Wq = np.asarray(Wqkv[:D], np.float32).reshape(H, DH, D)
    Wk = np.asarray(Wqkv[D:2 * D], np.float32).reshape(H, DH, D)
    Wv = np.asarray(Wqkv[2 * D:], np.float32).reshape(H, DH, D)
    WpT = np.ascontiguousarray(np.asarray(Wproj, np.float32).T)

    xT = []
    for b in range(B):
        xt = np.asarray(x[b], np.float32).T.astype(np.float16)  # [D, T]
        # strip-major: [128, (strip, kc, 512)] so each strip is one
        # contiguous 1MB DMA
        xT.append(np.ascontiguousarray(
            xt.reshape(KC, 128, TC512, 512).transpose(1, 2, 0, 3)
            .reshape(128, KC * T)))

    in_maps = []
    for c in range(NCORES):
        b, g = divmod(c, GROUPS)
        hs = [HPC * g + hl for hl in range(HPC)]
        cols = []
        for pp in range(NPAIR):
            h0, h1 = hs[2 * pp], hs[2 * pp + 1]
            cols.append(np.concatenate([Wq[h0], Wq[h1]], axis=0))
            cols.append(np.concatenate([Wk[h0], Wk[h1]], axis=0))
        wqk = _kc_fold(np.concatenate(cols, axis=0).T.astype(np.float16))
        wv = _kc_fold(np.concatenate(
            [Wv[h] for h in hs], axis=0).T.astype(np.float16))
        wp = np.ascontiguousarray(
            WpT[256 * g:256 * (g + 1), :].astype(np.float16))
        wp = np.ascontiguousarray(
            wp.reshape(2, 128, D).transpose(1, 0, 2).reshape(128, 2 * D))
        in_maps.append({
            "xT": xT[b], "wqk": wqk, "wv": wv, "wp": wp,
            "cs": cs, "msc": msc,
        })
    return in_maps


def _in_maps_for_trace(inputs):
    return _make_in_maps(np.asarray(inputs["x"]), np.asarray(inputs["Wqkv"]),
                         np.asarray(inputs["Wproj"]))


def kernel(x, attn_mask, key_padding_mask, Wqkv, Wproj):
    x = np.asarray(x)
    attn_mask = np.asarray(attn_mask)
    key_padding_mask = np.asarray(key_padding_mask)
    Wqkv = np.asarray(Wqkv)
    Wproj = np.asarray(Wproj)

    if not _canonical(attn_mask, key_padding_mask):
        return _reference_fallback(x, attn_mask, key_padding_mask, Wqkv, Wproj)

    nc = _get_module()
    in_maps = _make_in_maps(x, Wqkv, Wproj)
    res = bass_utils.run_bass_kernel_spmd(nc, in_maps,
                                          core_ids=list(range(NCORES)))
    out = np.empty((B, T, D), dtype=np.float32)
    for b in range(B):
        acc = res.results[4 * b]["o"].astype(np.float32)
        for g in range(1, GROUPS):
            acc += res.results[4 * b + g]["o"].astype(np.float32)
        # o is [128, (strip, oc, 512)]; out[b][512j+t', 128oc+p] = acc.T ...
        od = acc.reshape(128, TC512, 8, 512).transpose(2, 0, 1, 3)
        out[b] = od.reshape(D, T).T
    return out
